# revision 6
# baseline (speedup 1.0000x reference)
"""CBOW forward (mean-embed -> linear -> linear -> log_softmax) on 8 trn2 cores.

Vocab-parallel tensor parallelism: each core owns a V/8 = 4000-wide vocab shard
of the input slices, W1 columns, and W2 rows.  Layer-1 partial h is AllReduced
(64 KB), layer-2 + softmax statistics are computed shard-locally with a tiny
AllGather of per-core sum(exp(logits)).

Stage 1 fuses the context-mean and the [b,v] -> [v,b] transpose into one PE
pass per v-chunk via a constant selector matrix SM[p, j] = (p//8 == j)/8.
All matmuls run in bf16 (fp32 operands are ~2x slower per column on the PE and
disable fast-weight-load); accumulation stays fp32 in PSUM, and the softmax /
output path reads fp32 logits from PSUM.

Problem shapes (hardcoded): B=64, 2N=8 context slots, V=32000, D=256, fp32 IO.
"""

import numpy as np

import concourse.bacc as bacc
import concourse.mybir as mybir
import concourse.tile as tile
from concourse.bass_utils import run_bass_kernel_spmd

N_CORES = 8
B = 64          # batch
NCTX = 8        # 2N context slots
V = 32000
D = 256
VS = V // N_CORES          # 4000 vocab columns per core
VC = 128                   # main v-chunk width; 31 full chunks + one 32-tail
NFULL = VS // VC           # 31
VTAIL = VS - NFULL * VC    # 32
NVC = NFULL + 1            # 32 chunks total
ROWS = B * NCTX            # 512 input rows, row = b*NCTX + i
F32 = mybir.dt.float32
BF16 = mybir.dt.bfloat16

_cache = {}


def _build(dummy_cc=True):
    nc = bacc.Bacc("TRN2", target_bir_lowering=False, debug=False,
                   num_devices=N_CORES)

    X = nc.dram_tensor("x", [ROWS, VS], F32, kind="ExternalInput")
    W1T = nc.dram_tensor("w1t", [VS, D], F32, kind="ExternalInput")
    W2TB = nc.dram_tensor("w2tb", [D + 1, VS], F32, kind="ExternalInput")
    B1T = nc.dram_tensor("b1t", [128, 2], F32, kind="ExternalInput")
    SM = nc.dram_tensor("sm", [128, 16], BF16, kind="ExternalInput")
    I64 = nc.dram_tensor("i64", [64, 64], F32, kind="ExternalInput")
    OUT = nc.dram_tensor("out", [B, VS], F32, kind="ExternalOutput")

    rg = [list(range(N_CORES))]

    def vchunk(i):
        lo = i * VC
        return lo, (VTAIL if i == NFULL else VC)

    with tile.TileContext(nc) as tc:
        with (
            tc.tile_pool(name="consts", bufs=1) as consts,
            tc.tile_pool(name="xin", bufs=2) as xin,
            tc.tile_pool(name="xbf", bufs=2) as xbf,
            tc.tile_pool(name="wpool", bufs=1) as wpool,
            tc.tile_pool(name="work", bufs=1) as work,
            tc.tile_pool(name="dram", bufs=1, space="DRAM") as dram,
        ):
            # Warmup collective: absorbs the cross-core launch barrier and
            # first-collective setup cost while stage-1 DMA/compute runs.
            if dummy_cc:
                warm_sb = consts.tile([1, 16], F32)
                nc.gpsimd.memset(warm_sb[:], 0.0)
                warm_in = dram.tile([1, 16], F32)
                warm_out = dram.tile([N_CORES, 16], F32, addr_space="Shared")
                nc.sync.dma_start(warm_in[:], warm_sb[:])
                nc.gpsimd.collective_compute(
                    "AllGather", mybir.AluOpType.bypass, replica_groups=rg,
                    ins=[warm_in.opt()], outs=[warm_out.opt()])

            sm_sb = consts.tile([128, 16], BF16)
            nc.sync.dma_start(sm_sb[:], SM.ap())
            i64_sb = consts.tile([64, 64], F32)
            nc.sync.dma_start(i64_sb[:], I64.ap())
            b1_sb = consts.tile([128, 2], F32)
            nc.sync.dma_start(b1_sb[:], B1T.ap())
            ones_sb = consts.tile([1, 64], BF16)
            nc.gpsimd.memset(ones_sb[:], 1.0)

            # Phase 1: x_bar^T[v, b] = mean_i X[b, i, v], fused transpose+mean
            # on PE.  X tile t holds rows 128t..128t+127 = b in [16t, 16t+16).
            xbar_sb = work.tile([128, NVC * B], BF16)
            with tc.tile_pool(name="ps1", bufs=1, space="PSUM") as ps1:
                xbar_ps = ps1.tile([128, NVC * B], F32)   # 4 banks
                h_ps = ps1.tile([B, D], F32)              # 1 bank

                w1t_sb = w1t_bf = w1tail_sb = w1tail_bf = None
                for t in range(4):
                    xt = xin.tile([128, VS], F32, tag="xt")
                    nc.sync.dma_start(xt[:], X.ap()[128 * t:128 * (t + 1), :])
                    xb = xbf.tile([128, VS], BF16, tag="xb")
                    nc.scalar.copy(xb[:], xt[:])
                    if t == 0:
                        # W1T load + DVE cast: issued after X0 so stage 1
                        # starts ASAP; ready well before layer 1.
                        w1t_sb = wpool.tile([VC, NFULL, D], F32)
                        nc.sync.dma_start(
                            w1t_sb[:],
                            W1T.ap()[0:NFULL * VC, :].rearrange(
                                "(c p) d -> p c d", p=VC))
                        w1tail_sb = wpool.tile([VTAIL, D], F32)
                        nc.sync.dma_start(
                            w1tail_sb[:], W1T.ap()[NFULL * VC:VS, :])
                        w1t_bf = wpool.tile([VC, NFULL, D], BF16)
                        nc.vector.tensor_copy(w1t_bf[:], w1t_sb[:])
                        w1tail_bf = wpool.tile([VTAIL, D], BF16)
                        nc.vector.tensor_copy(w1tail_bf[:], w1tail_sb[:])
                    for i in range(NVC):
                        lo, w = vchunk(i)
                        nc.tensor.matmul(
                            xbar_ps[0:w, i * B + 16 * t: i * B + 16 * (t + 1)],
                            xb[:, lo:lo + w],
                            sm_sb[:],
                            start=True, stop=True,
                        )

                # Phase 2: h[b, d] = sum_v xbar^T[v, b] * W1T[v, d]
                for i in range(NVC):
                    lo, w = vchunk(i)
                    nc.vector.tensor_copy(
                        xbar_sb[0:w, i * B:(i + 1) * B],
                        xbar_ps[0:w, i * B:(i + 1) * B])
                    rhs = (w1t_bf[:, i, :] if i < NFULL else w1tail_bf[:])
                    nc.tensor.matmul(
                        h_ps[:],
                        xbar_sb[0:w, i * B:(i + 1) * B],
                        rhs,
                        start=(i == 0), stop=(i == NVC - 1),
                    )

                h_sb = work.tile([B, D], F32)
                nc.vector.tensor_copy(h_sb[:], h_ps[:])

            # AllReduce partial h across the 8 vocab shards.
            hb_in = dram.tile([B, D], F32)
            hb_out = dram.tile([B, D], F32, addr_space="Shared")
            nc.sync.dma_start(hb_in[:], h_sb[:])
            nc.gpsimd.collective_compute(
                "AllReduce", mybir.AluOpType.add, replica_groups=rg,
                ins=[hb_in.opt()], outs=[hb_out.opt()])
            hsum_sb = work.tile([B, D], F32)
            nc.sync.dma_start(hsum_sb[:], hb_out[:])

            # W2 + b2 load/cast: emitted after the AR trigger so the gpsimd
            # queue fires the collective first; runs during the AR wait.
            w2_sb = wpool.tile([128, 2, VS], F32)
            nc.sync.dma_start(
                w2_sb[:], W2TB.ap()[0:D, :].rearrange("(c p) n -> p c n", p=128))
            b2_sb = wpool.tile([1, VS], F32)
            nc.sync.dma_start(b2_sb[:], W2TB.ap()[D:D + 1, :])
            w2_bf = wpool.tile([128, 2, VS], BF16)
            nc.vector.tensor_copy(w2_bf[:], w2_sb[:])
            b2_bf = wpool.tile([1, VS], BF16)
            nc.vector.tensor_copy(b2_bf[:], b2_sb[:])

            # h^T[d, b] via PE transpose, + b1 fused into the PSUM->SBUF copy
            # (cast to bf16 for layer 2).
            hT_sb = work.tile([128, 2, B], BF16)
            with tc.tile_pool(name="ps2", bufs=1, space="PSUM") as ps2:
                for dc in range(2):
                    hT_ps = ps2.tile([128, B], F32, tag="hT")
                    nc.tensor.transpose(
                        hT_ps[:], hsum_sb[:, dc * 128:(dc + 1) * 128], i64_sb[:])
                    nc.vector.tensor_scalar_add(
                        hT_sb[:, dc, :], hT_ps[:], b1_sb[:, dc:dc + 1])

            # Layer 2 + log-softmax.
            e_sb = work.tile([B, VS], F32)
            out_sb = work.tile([B, VS], F32)
            sumexp_sb = work.tile([B, 1], F32)

            with tc.tile_pool(name="ps3", bufs=1, space="PSUM") as ps3:
                logits_ps = ps3.tile([B, 4096], F32)      # 8 banks
                nsplits = [(k * 512, min(512, VS - k * 512)) for k in range(8)]
                for n0, nw in nsplits:
                    for dc in range(2):
                        nc.tensor.matmul(
                            logits_ps[:, n0:n0 + nw],
                            hT_sb[:, dc, :],
                            w2_bf[:, dc, n0:n0 + nw],
                            start=(dc == 0), stop=False,
                        )
                    nc.tensor.matmul(
                        logits_ps[:, n0:n0 + nw],
                        ones_sb[:],
                        b2_bf[:, n0:n0 + nw],
                        start=False, stop=True,
                    )

                # sum(exp(logits)) per row in one ACT pass; logits are O(+-3)
                # so no max-subtraction is needed in fp32.
                nc.scalar.activation(
                    e_sb[:], logits_ps[:, 0:VS],
                    mybir.ActivationFunctionType.Exp,
                    accum_out=sumexp_sb[:])

                # Global sumexp: AllGather the 8 per-core partial sums.
                sb_in = dram.tile([B, 1], F32)
                sb_out = dram.tile([N_CORES, B], F32, addr_space="Shared")
                nc.sync.dma_start(sb_in[:], sumexp_sb[:])
                nc.gpsimd.collective_compute(
                    "AllGather", mybir.AluOpType.bypass, replica_groups=rg,
                    ins=[sb_in.opt()], outs=[sb_out.opt()])
                sg_sb = work.tile([B, N_CORES], F32)
                nc.sync.dma_start(sg_sb[:], sb_out[:].rearrange("r b -> b r"))

                stot_sb = work.tile([B, 1], F32)
                nc.vector.reduce_sum(stot_sb[:], sg_sb[:],
                                     axis=mybir.AxisListType.X)
                logs_sb = work.tile([B, 1], F32)
                nc.scalar.activation(logs_sb[:], stot_sb[:],
                                     mybir.ActivationFunctionType.Ln)
                neglogs_sb = work.tile([B, 1], F32)
                nc.vector.tensor_scalar_mul(neglogs_sb[:], logs_sb[:], -1.0)

                # out = logits - log(sumexp): halves split across DVE and ACT,
                # output DMA chunked to overlap.
                H = VS // 2
                nc.vector.tensor_scalar_sub(
                    out_sb[:, 0:H], logits_ps[:, 0:H], logs_sb[:])
                nc.scalar.activation(
                    out_sb[:, H:VS], logits_ps[:, H:VS],
                    mybir.ActivationFunctionType.Identity,
                    bias=neglogs_sb[:])
                nc.sync.dma_start(OUT.ap()[:, 0:H], out_sb[:, 0:H])
                nc.sync.dma_start(OUT.ap()[:, H:VS], out_sb[:, H:VS])

    nc.compile()
    return nc


def _get_nc():
    if "nc" not in _cache:
        _cache["nc"] = _build()
    return _cache["nc"]


def _make_in_maps(input_vec, W1, b1, W2, b2):
    input_vec = np.asarray(input_vec, dtype=np.float32)
    W1 = np.asarray(W1, dtype=np.float32)
    b1 = np.asarray(b1, dtype=np.float32)
    W2 = np.asarray(W2, dtype=np.float32)
    b2 = np.asarray(b2, dtype=np.float32)

    import ml_dtypes

    xr = input_vec.reshape(B, NCTX, V)
    sm = (np.repeat(np.eye(16, dtype=np.float32), NCTX, axis=0) / NCTX)
    sm = sm.astype(ml_dtypes.bfloat16)
    i64 = np.eye(64, dtype=np.float32)
    b1t = np.ascontiguousarray(b1.reshape(2, 128).T)

    in_maps = []
    for c in range(N_CORES):
        lo, hi = c * VS, (c + 1) * VS
        xc = np.ascontiguousarray(xr[:, :, lo:hi]).reshape(ROWS, VS)
        w1t = np.ascontiguousarray(W1[:, lo:hi].T)
        w2tb = np.concatenate(
            [np.ascontiguousarray(W2[lo:hi, :].T), b2[None, lo:hi]], axis=0)
        in_maps.append({
            "x": xc, "w1t": w1t, "w2tb": np.ascontiguousarray(w2tb),
            "b1t": b1t, "sm": sm, "i64": i64,
        })
    return in_maps


def kernel(input_vec, W1, b1, W2, b2, **_unused):
    in_maps = _make_in_maps(input_vec, W1, b1, W2, b2)
    _cache["in_maps"] = in_maps
    nc = _get_nc()
    res = run_bass_kernel_spmd(nc, in_maps, core_ids=list(range(N_CORES)))
    return np.concatenate([res.results[c]["out"] for c in range(N_CORES)],
                          axis=1)


# revision 7
# speedup vs baseline: 1.0051x; 1.0051x over previous
"""CBOW forward (mean-embed -> linear -> linear -> log_softmax) on 8 trn2 cores.

Vocab-parallel tensor parallelism: each core owns a V/8 = 4000-wide vocab shard
of the input slices, W1 columns, and W2 rows.  Layer-1 partial h is AllReduced
(64 KB), layer-2 + softmax statistics are computed shard-locally with a tiny
AllGather of per-core sum(exp(logits)).

Stage 1 fuses the context-mean and the [b,v] -> [v,b] transpose into one PE
pass per v-chunk via a constant selector matrix SM[p, j] = (p//8 == j)/8.
All matmuls run in bf16 (fp32 operands are ~2x slower per column on the PE and
disable fast-weight-load); accumulation stays fp32 in PSUM, and the softmax /
output path reads fp32 logits from PSUM.

Problem shapes (hardcoded): B=64, 2N=8 context slots, V=32000, D=256, fp32 IO.
"""

import numpy as np

import concourse.bacc as bacc
import concourse.mybir as mybir
import concourse.tile as tile
from concourse.bass_utils import run_bass_kernel_spmd

N_CORES = 8
B = 64          # batch
NCTX = 8        # 2N context slots
V = 32000
D = 256
VS = V // N_CORES          # 4000 vocab columns per core
VC = 128                   # main v-chunk width; 31 full chunks + one 32-tail
NFULL = VS // VC           # 31
VTAIL = VS - NFULL * VC    # 32
NVC = NFULL + 1            # 32 chunks total
ROWS = B * NCTX            # 512 input rows, row = b*NCTX + i
F32 = mybir.dt.float32
BF16 = mybir.dt.bfloat16

_cache = {}


def _build(dummy_cc=True):
    nc = bacc.Bacc("TRN2", target_bir_lowering=False, debug=False,
                   num_devices=N_CORES)

    X = nc.dram_tensor("x", [ROWS, VS], F32, kind="ExternalInput")
    W1TP = nc.dram_tensor("w1tp", [128, NVC, D], F32, kind="ExternalInput")
    W2TP = nc.dram_tensor("w2tp", [128, 2, VS], F32, kind="ExternalInput")
    B2 = nc.dram_tensor("b2", [1, VS], F32, kind="ExternalInput")
    B1T = nc.dram_tensor("b1t", [128, 2], F32, kind="ExternalInput")
    SM = nc.dram_tensor("sm", [128, 16], BF16, kind="ExternalInput")
    I64 = nc.dram_tensor("i64", [64, 64], F32, kind="ExternalInput")
    OUT = nc.dram_tensor("out", [B, VS], F32, kind="ExternalOutput")

    rg = [list(range(N_CORES))]

    def vchunk(i):
        lo = i * VC
        return lo, (VTAIL if i == NFULL else VC)

    with tile.TileContext(nc) as tc:
        with (
            tc.tile_pool(name="consts", bufs=1) as consts,
            tc.tile_pool(name="xin", bufs=2) as xin,
            tc.tile_pool(name="xbf", bufs=2) as xbf,
            tc.tile_pool(name="wpool", bufs=1) as wpool,
            tc.tile_pool(name="work", bufs=1) as work,
            tc.tile_pool(name="dram", bufs=1, space="DRAM") as dram,
        ):
            # Warmup collective: absorbs the cross-core launch barrier and
            # first-collective setup cost while stage-1 DMA/compute runs.
            if dummy_cc:
                warm_sb = consts.tile([1, 16], F32)
                nc.gpsimd.memset(warm_sb[:], 0.0)
                warm_in = dram.tile([1, 16], F32)
                warm_out = dram.tile([N_CORES, 16], F32, addr_space="Shared")
                nc.sync.dma_start(warm_in[:], warm_sb[:])
                nc.gpsimd.collective_compute(
                    "AllGather", mybir.AluOpType.bypass, replica_groups=rg,
                    ins=[warm_in.opt()], outs=[warm_out.opt()])

            sm_sb = consts.tile([128, 16], BF16)
            nc.sync.dma_start(sm_sb[:], SM.ap())
            i64_sb = consts.tile([64, 64], F32)
            nc.sync.dma_start(i64_sb[:], I64.ap())
            b1_sb = consts.tile([128, 2], F32)
            nc.sync.dma_start(b1_sb[:], B1T.ap())
            ones_sb = consts.tile([1, 64], BF16)
            nc.gpsimd.memset(ones_sb[:], 1.0)

            # Phase 1: x_bar^T[v, b] = mean_i X[b, i, v], fused transpose+mean
            # on PE.  X tile t holds rows 128t..128t+127 = b in [16t, 16t+16).
            xbar_sb = work.tile([128, NVC * B], BF16)
            with tc.tile_pool(name="ps1", bufs=1, space="PSUM") as ps1:
                xbar_ps = ps1.tile([128, NVC * B], F32)   # 4 banks
                h_ps = ps1.tile([B, D], F32)              # 1 bank

                w1t_sb = w1t_bf = w1tail_sb = w1tail_bf = None
                for t in range(4):
                    xt = xin.tile([128, VS], F32, tag="xt")
                    nc.sync.dma_start(xt[:], X.ap()[128 * t:128 * (t + 1), :])
                    xb = xbf.tile([128, VS], BF16, tag="xb")
                    for c0 in range(0, VS, 1024):
                        c1 = min(c0 + 1024, VS)
                        nc.scalar.copy(xb[:, c0:c1], xt[:, c0:c1])
                    if t == 0:
                        # W1T load (host pre-arranged [128, chunk, d], so the
                        # DMA is contiguous) + DVE cast; ready before layer 1.
                        w1t_sb = wpool.tile([128, NVC, D], F32)
                        nc.sync.dma_start(w1t_sb[:], W1TP.ap())
                        w1t_bf = wpool.tile([128, NVC, D], BF16)
                        nc.vector.tensor_copy(w1t_bf[:], w1t_sb[:])
                    for i in range(NVC):
                        lo, w = vchunk(i)
                        nc.tensor.matmul(
                            xbar_ps[0:w, i * B + 16 * t: i * B + 16 * (t + 1)],
                            xb[:, lo:lo + w],
                            sm_sb[:],
                            start=True, stop=True,
                        )

                # Phase 2: h[b, d] = sum_v xbar^T[v, b] * W1T[v, d]
                for i in range(NVC):
                    lo, w = vchunk(i)
                    nc.vector.tensor_copy(
                        xbar_sb[0:w, i * B:(i + 1) * B],
                        xbar_ps[0:w, i * B:(i + 1) * B])
                    rhs = w1t_bf[0:w, i, :]
                    nc.tensor.matmul(
                        h_ps[:],
                        xbar_sb[0:w, i * B:(i + 1) * B],
                        rhs,
                        start=(i == 0), stop=(i == NVC - 1),
                    )

                h_sb = work.tile([B, D], F32)
                nc.vector.tensor_copy(h_sb[:], h_ps[:])

            # AllReduce partial h across the 8 vocab shards.
            hb_in = dram.tile([B, D], F32)
            hb_out = dram.tile([B, D], F32, addr_space="Shared")
            nc.sync.dma_start(hb_in[:], h_sb[:])
            nc.gpsimd.collective_compute(
                "AllReduce", mybir.AluOpType.add, replica_groups=rg,
                ins=[hb_in.opt()], outs=[hb_out.opt()])
            hsum_sb = work.tile([B, D], F32)
            nc.sync.dma_start(hsum_sb[:], hb_out[:])

            # W2 + b2 load/cast: on the gpsimd SWDGE queue so the critical
            # hsum DMA on the sync queue is not stuck behind 4 MB; runs
            # during the AR wait.
            w2_sb = wpool.tile([128, 2, VS], F32)
            nc.gpsimd.dma_start(w2_sb[:], W2TP.ap())
            b2_sb = wpool.tile([1, VS], F32)
            nc.gpsimd.dma_start(b2_sb[:], B2.ap())
            w2_bf = wpool.tile([128, 2, VS], BF16)
            nc.vector.tensor_copy(w2_bf[:], w2_sb[:])
            b2_bf = wpool.tile([1, VS], BF16)
            nc.vector.tensor_copy(b2_bf[:], b2_sb[:])

            # h^T[d, b] via PE transpose, + b1 fused into the PSUM->SBUF copy
            # (cast to bf16 for layer 2).
            hT_sb = work.tile([128, 2, B], BF16)
            with tc.tile_pool(name="ps2", bufs=1, space="PSUM") as ps2:
                for dc in range(2):
                    hT_ps = ps2.tile([128, B], F32, tag="hT")
                    nc.tensor.transpose(
                        hT_ps[:], hsum_sb[:, dc * 128:(dc + 1) * 128], i64_sb[:])
                    nc.vector.tensor_scalar_add(
                        hT_sb[:, dc, :], hT_ps[:], b1_sb[:, dc:dc + 1])

            # Layer 2 + log-softmax.
            e_sb = work.tile([B, VS], F32)
            out_sb = work.tile([B, VS], F32)
            sumexp_sb = work.tile([B, 1], F32)

            with tc.tile_pool(name="ps3", bufs=1, space="PSUM") as ps3:
                logits_ps = ps3.tile([B, 4096], F32)      # 8 banks
                nsplits = [(k * 512, min(512, VS - k * 512)) for k in range(8)]
                for n0, nw in nsplits:
                    for dc in range(2):
                        nc.tensor.matmul(
                            logits_ps[:, n0:n0 + nw],
                            hT_sb[:, dc, :],
                            w2_bf[:, dc, n0:n0 + nw],
                            start=(dc == 0), stop=False,
                        )
                    nc.tensor.matmul(
                        logits_ps[:, n0:n0 + nw],
                        ones_sb[:],
                        b2_bf[:, n0:n0 + nw],
                        start=False, stop=True,
                    )

                # sum(exp(logits)) per row, one ACT pass per PSUM bank so
                # exp overlaps the remaining layer-2 matmuls; logits are
                # O(+-3) so no max-subtraction is needed in fp32.
                sums8_sb = work.tile([B, 8], F32)
                for k, (n0, nw) in enumerate(nsplits):
                    nc.scalar.activation(
                        e_sb[:, n0:n0 + nw], logits_ps[:, n0:n0 + nw],
                        mybir.ActivationFunctionType.Exp,
                        accum_out=sums8_sb[:, k:k + 1])
                nc.vector.reduce_sum(sumexp_sb[:], sums8_sb[:],
                                     axis=mybir.AxisListType.X)

                # Global sumexp: AllGather the 8 per-core partial sums.
                sb_in = dram.tile([B, 1], F32)
                sb_out = dram.tile([N_CORES, B], F32, addr_space="Shared")
                nc.sync.dma_start(sb_in[:], sumexp_sb[:])
                nc.gpsimd.collective_compute(
                    "AllGather", mybir.AluOpType.bypass, replica_groups=rg,
                    ins=[sb_in.opt()], outs=[sb_out.opt()])
                sg_sb = work.tile([B, N_CORES], F32)
                nc.sync.dma_start(sg_sb[:], sb_out[:].rearrange("r b -> b r"))

                stot_sb = work.tile([B, 1], F32)
                nc.vector.reduce_sum(stot_sb[:], sg_sb[:],
                                     axis=mybir.AxisListType.X)
                logs_sb = work.tile([B, 1], F32)
                nc.scalar.activation(logs_sb[:], stot_sb[:],
                                     mybir.ActivationFunctionType.Ln)
                neglogs_sb = work.tile([B, 1], F32)
                nc.vector.tensor_scalar_mul(neglogs_sb[:], logs_sb[:], -1.0)

                # out = logits - log(sumexp): halves split across DVE and ACT,
                # output DMA chunked to overlap.
                H = VS // 2
                nc.vector.tensor_scalar_sub(
                    out_sb[:, 0:H], logits_ps[:, 0:H], logs_sb[:])
                nc.scalar.activation(
                    out_sb[:, H:VS], logits_ps[:, H:VS],
                    mybir.ActivationFunctionType.Identity,
                    bias=neglogs_sb[:])
                nc.sync.dma_start(OUT.ap()[:, 0:H], out_sb[:, 0:H])
                nc.sync.dma_start(OUT.ap()[:, H:VS], out_sb[:, H:VS])

    nc.compile()
    return nc


def _get_nc():
    if "nc" not in _cache:
        _cache["nc"] = _build()
    return _cache["nc"]


def _make_in_maps(input_vec, W1, b1, W2, b2):
    input_vec = np.asarray(input_vec, dtype=np.float32)
    W1 = np.asarray(W1, dtype=np.float32)
    b1 = np.asarray(b1, dtype=np.float32)
    W2 = np.asarray(W2, dtype=np.float32)
    b2 = np.asarray(b2, dtype=np.float32)

    import ml_dtypes

    xr = input_vec.reshape(B, NCTX, V)
    sm = (np.repeat(np.eye(16, dtype=np.float32), NCTX, axis=0) / NCTX)
    sm = sm.astype(ml_dtypes.bfloat16)
    i64 = np.eye(64, dtype=np.float32)
    b1t = np.ascontiguousarray(b1.reshape(2, 128).T)

    in_maps = []
    for c in range(N_CORES):
        lo, hi = c * VS, (c + 1) * VS
        xc = np.ascontiguousarray(xr[:, :, lo:hi]).reshape(ROWS, VS)
        w1s = W1[:, lo:hi].T                       # [VS, D]
        w1tp = np.zeros((128, NVC, D), np.float32)
        w1tp[:, :NFULL, :] = w1s[:NFULL * VC].reshape(NFULL, VC, D).transpose(1, 0, 2)
        w1tp[:VTAIL, NFULL, :] = w1s[NFULL * VC:]
        w2tp = np.ascontiguousarray(
            W2[lo:hi, :].T.reshape(2, 128, VS).transpose(1, 0, 2))
        in_maps.append({
            "x": xc, "w1tp": w1tp, "w2tp": w2tp,
            "b2": np.ascontiguousarray(b2[None, lo:hi]),
            "b1t": b1t, "sm": sm, "i64": i64,
        })
    return in_maps


def kernel(input_vec, W1, b1, W2, b2, **_unused):
    in_maps = _make_in_maps(input_vec, W1, b1, W2, b2)
    _cache["in_maps"] = in_maps
    nc = _get_nc()
    res = run_bass_kernel_spmd(nc, in_maps, core_ids=list(range(N_CORES)))
    return np.concatenate([res.results[c]["out"] for c in range(N_CORES)],
                          axis=1)
